# revision 22
# baseline (speedup 1.0000x reference)
"""CVQNN batched policy forward on 8 Trainium2 NeuronCores (fp16 v2).

Math: B=256 4-mode Fock states (cutoff 10) through 4 CVQNN layers.
Device layout per core: state [100 partitions = mode pair (maj,min),
6400 free = fA*640 + plane*320 + fB*32 + b], fp16, complex as separate
re/im planes nested INSIDE each fA block so a pair-digit shuffle is a
contiguous 640-element move.

Every gate is a 100x100 complex matmul on the partition-side mode pair
(4 fp16 matmuls with PSUM accumulation, 640-wide chunks, 3 LDWEIGHTS
per gate via weight-reuse ordering). Diagonal/single-mode gates are
folded into pair gates on the host. Pair-layout changes: 'S' = 10
SBUF->SBUF DMAs (1280B descriptors), 'F' = PE transposes. Batch is
data-parallel over 8 cores.
"""
import numpy as np

B, D, CUT, LAYERS, NCORES = 256, 4, 10, 4, 8
K = D * (D - 1) // 2
C2 = CUT * CUT
BC = B // NCORES            # 32 batch per core
# free strides (elements): fA*640 + plane*320 + fB*32 + b
FTOT = 2 * C2 * BC          # 6400 free elems per state tile
PF = C2 * BC                # 3200 elems per plane (logical)
NMM = 12 * LAYERS + 1       # 49 pair-gate matmuls

# ---------------------------------------------------------------- host math

def _ops():
    a = np.diag(np.sqrt(np.arange(1, CUT)), k=1).astype(np.complex128)
    return a, a.conj().T.copy(), np.arange(CUT, dtype=np.float64)


def _expm_antiherm(G):
    w, V = np.linalg.eigh(-1j * G)
    return (V * np.exp(1j * w)) @ V.conj().T


def _bs(a, adag, t, p):
    Aab = np.kron(adag, a)
    return _expm_antiherm(t * (np.exp(1j * p) * Aab - np.exp(-1j * p) * Aab.conj().T))


def _orient(M, pair, in_order, out_order):
    """M acts canonically on index m_x*10+m_y for pair=(x,y).  Reindex for
    input digit order in_order and output digit order out_order."""
    M4 = M.reshape(CUT, CUT, CUT, CUT)  # [ox, oy, ix, iy]
    perm = [0, 1, 2, 3]
    if tuple(out_order) != tuple(pair):
        perm[0], perm[1] = perm[1], perm[0]
    if tuple(in_order) != tuple(pair):
        perm[2], perm[3] = perm[3], perm[2]
    return M4.transpose(perm).reshape(C2, C2)


def build_gates(cvqnn_weights):
    """Returns mm_list where entries are (matrix[100x100 complex],
    swap_free: bool). Layout walk is fixed."""
    a, adag, n = _ops()
    I10 = np.eye(CUT, dtype=np.complex128)
    w = np.asarray(cvqnn_weights, np.float64)
    PAIRS = [(0, 1), (0, 2), (0, 3), (1, 2), (1, 3), (2, 3)]

    mms = []
    fold01 = np.eye(C2, dtype=np.complex128)
    fold2 = I10.copy()
    fold3 = I10.copy()
    for l in range(LAYERS):
        o = 0
        th1 = w[l, o:o + K]; o += K
        ph1 = w[l, o:o + K]; o += K
        vp1 = w[l, o:o + D]; o += D
        rsq = w[l, o:o + D]; o += D
        th2 = w[l, o:o + K]; o += K
        ph2 = w[l, o:o + K]; o += K
        vp2 = w[l, o:o + D]; o += D
        rd = w[l, o:o + D]; o += D
        phd = w[l, o:o + D]; o += D
        kap = w[l, o:o + D]
        U = {PAIRS[q]: _bs(a, adag, th1[q], ph1[q]) for q in range(K)}
        V = {PAIRS[q]: _bs(a, adag, th2[q], ph2[q]) for q in range(K)}
        S = [_expm_antiherm(0.5 * rsq[m] * (a @ a - adag @ adag)) for m in range(D)]
        al = rd * np.exp(1j * phd)
        Dm = [_expm_antiherm(al[m] * adag - np.conj(al[m]) * a) for m in range(D)]
        P1 = [np.diag(np.exp(1j * vp1[m] * n)) for m in range(D)]
        P2 = [np.diag(np.exp(1j * vp2[m] * n)) for m in range(D)]
        Km = [np.diag(np.exp(1j * kap[m] * n * n)) for m in range(D)]
        SQ01 = np.kron(S[0] @ P1[0], S[1] @ P1[1])
        SQ23 = np.kron(S[2] @ P1[2], S[3] @ P1[3])
        DP01 = np.kron(Dm[0] @ P2[0], Dm[1] @ P2[1])
        DP23 = np.kron(Dm[2] @ P2[2], Dm[3] @ P2[3])

        mms.append((_orient(U[(0, 1)] @ fold01, (0, 1), (1, 0), (1, 0)), False))
        mms.append((_orient(U[(0, 2)] @ np.kron(I10, fold2), (0, 2), (0, 2), (2, 0)), True))
        mms.append((_orient(U[(0, 3)] @ np.kron(I10, fold3), (0, 3), (0, 3), (0, 3)), False))
        mms.append((_orient(U[(1, 2)], (1, 2), (2, 1), (2, 1)), False))
        mms.append((_orient(U[(1, 3)], (1, 3), (1, 3), (1, 3)), False))
        mms.append((_orient(SQ23 @ U[(2, 3)], (2, 3), (3, 2), (3, 2)), False))
        mms.append((_orient(V[(0, 1)] @ SQ01, (0, 1), (1, 0), (1, 0)), False))
        mms.append((_orient(V[(0, 2)], (0, 2), (0, 2), (2, 0)), True))
        mms.append((_orient(V[(0, 3)], (0, 3), (0, 3), (0, 3)), False))
        mms.append((_orient(V[(1, 2)], (1, 2), (2, 1), (2, 1)), False))
        mms.append((_orient(V[(1, 3)], (1, 3), (1, 3), (1, 3)), False))
        mms.append((_orient(DP23 @ V[(2, 3)], (2, 3), (3, 2), (3, 2)), False))
        if l < LAYERS - 1:
            fold01 = np.kron(Km[0], Km[1]) @ DP01
            fold2 = Km[2]
            fold3 = Km[3]
        else:
            mms.append((_orient(DP01, (0, 1), (1, 0), (1, 0)), False))
    assert len(mms) == NMM
    return mms


# op schedule per layer: 'M' gate, 'S' shuffle, 'F' flip
LAYER_OPS = ['M', 'S', 'M', 'S', 'M', 'F', 'M', 'S', 'M', 'S', 'M', 'F',
             'M', 'S', 'M', 'S', 'M', 'F', 'M', 'S', 'M', 'S', 'M', 'F']
FULL_OPS = LAYER_OPS * LAYERS + ['M']

MM_SWAP = []
for _l in range(LAYERS):
    MM_SWAP += [False, True, False, False, False, False,
                False, True, False, False, False, False]
MM_SWAP.append(False)


def initial_state_dev(inputs):
    """Per-core device state tiles [NCORES, 100, 6400] fp16 in layout
    [1,0 | 2,3]: p=m1*10+m0, f=fA(m2)*640 + plane*320 + fB(m3)*32 + b."""
    a, adag, n = _ops()
    z = 0.5j
    S0 = _expm_antiherm(0.5 * (np.conj(z) * (a @ a) - z * (adag @ adag)))
    psi0 = S0[:, 0]
    r = np.asarray(inputs, np.float64).reshape(-1)
    wv, Vx = np.linalg.eigh(-1j * (adag - a))
    w0 = Vx.conj().T @ psi0
    psi = (np.exp(1j * np.outer(r, wv)) * w0[None, :]) @ Vx.T
    psi = psi.reshape(B, D, CUT)
    st = np.einsum('bi,bj,bk,bl->bijkl', psi[:, 0], psi[:, 1], psi[:, 2], psi[:, 3])
    # [b, m0,m1,m2,m3] -> p=(m1,m0), fA=m2, fB=m3
    st = st.transpose(2, 1, 3, 4, 0).reshape(C2, CUT, CUT, B)  # [p, fA, fB, b]
    out = np.empty((NCORES, C2, CUT, 2, CUT, BC), np.float32)
    for c in range(NCORES):
        blk = st[:, :, :, c * BC:(c + 1) * BC]  # [100, 10, 10, 32]
        out[c, :, :, 0, :, :] = blk.real
        out[c, :, :, 1, :, :] = blk.imag
    return out.reshape(NCORES, C2, FTOT).astype(np.float16)


def readout_weights():
    """lhsT [100, 4] fp16 for the device readout matmul, layout [1,0|2,3]."""
    n = np.arange(CUT, dtype=np.float32)
    Wt = np.zeros((C2, 4), np.float32)
    for p in range(C2):
        Wt[p, 0] = n[p % 10]    # mode 0 (partition minor)
        Wt[p, 1] = n[p // 10]   # mode 1 (partition major)
        Wt[p, 2] = 1.0
        Wt[p, 3] = 1.0
    return Wt.astype(np.float16)


def assemble_output(routs):
    """routs: [NCORES, 4, 3200] f32 (free = fA*320 + fB*32 + b) -> [B, 4]."""
    n = np.arange(CUT, dtype=np.float64)
    out = np.zeros((B, D), np.float64)
    for c in range(NCORES):
        R = np.asarray(routs[c], np.float64).reshape(4, CUT, CUT, BC)
        sl = slice(c * BC, (c + 1) * BC)
        out[sl, 0] = R[0].sum(axis=(0, 1))
        out[sl, 1] = R[1].sum(axis=(0, 1))
        out[sl, 2] = (R[2] * n[:, None, None]).sum(axis=(0, 1))   # weight by fA=m2
        out[sl, 3] = (R[3] * n[None, :, None]).sum(axis=(0, 1))   # weight by fB=m3
    return out.astype(np.float32)


GW = 128   # gate stationary padded to 128 cols (fast weight load)


def gates_dram(mms):
    """[100, NMM*3*GW] fp16: per gate UrT | (-Ui)T | UiT, each zero-padded
    to 128 columns so LDWEIGHTS takes the fast path."""
    g = np.zeros((C2, NMM * 3 * GW), np.float32)
    for i, (M, _) in enumerate(mms):
        g[:, i * 3 * GW:i * 3 * GW + C2] = M.real.T.astype(np.float32)
        g[:, i * 3 * GW + GW:i * 3 * GW + GW + C2] = (-M.imag.T).astype(np.float32)
        g[:, i * 3 * GW + 2 * GW:i * 3 * GW + 2 * GW + C2] = M.imag.T.astype(np.float32)
    return g.astype(np.float16)


def make_in_maps(st, gd):
    ident = np.eye(C2, dtype=np.float16)
    wr = readout_weights()
    return [{"state0": st[c], "gates": gd, "ident": ident, "wread": wr}
            for c in range(NCORES)]


# ------------------------------------------------------------ numpy dev-sim

def dev_sim(state_core, mms):
    """Numpy model of the device op stream for one core (fp16 rounding).
    state_core: [100, 6400] f16. Returns R [4, 3200] f32."""
    f64 = np.float64
    rnd = lambda x: x.astype(np.float16).astype(f64)
    X = state_core.astype(f64).reshape(C2, CUT, 2, CUT, BC)  # [p, fA, pl, fB, b]
    mi = 0
    for op in FULL_OPS:
        if op == 'M':
            M, swap = mms[mi]; mi += 1
            Mr = rnd(M.real); Mi = rnd(M.imag)
            re = X[:, :, 0]; im = X[:, :, 1]          # [p, fA, fB, b]
            re2 = np.tensordot(Mr, re, axes=(1, 0)) - np.tensordot(Mi, im, axes=(1, 0))
            im2 = np.tensordot(Mr, im, axes=(1, 0)) + np.tensordot(Mi, re, axes=(1, 0))
            re2 = rnd(re2); im2 = rnd(im2)
            if swap:
                re2 = re2.transpose(0, 2, 1, 3)
                im2 = im2.transpose(0, 2, 1, 3)
            X = np.stack([re2, im2], axis=2).copy()
        elif op == 'S':
            # [v,s | u, pl, w, b] -> [s,u | v, pl, w, b]
            X6 = X.reshape(CUT, CUT, CUT, 2, CUT, BC)  # [v, s, u, pl, w, b]
            X = X6.transpose(1, 2, 0, 3, 4, 5).reshape(C2, CUT, 2, CUT, BC)
        else:  # F
            # [p1,p2 | f1, pl, f2, b] -> [f1,f2 | p2, pl, p1, b]
            X6 = X.reshape(CUT, CUT, CUT, 2, CUT, BC)  # [p1, p2, f1, pl, f2, b]
            X = X6.transpose(2, 4, 1, 3, 0, 5).reshape(C2, CUT, 2, CUT, BC)
    P = X[:, :, 0] ** 2 + X[:, :, 1] ** 2              # [p, fA, fB, b]
    Wt = readout_weights().astype(f64)
    return np.tensordot(Wt.T, P.reshape(C2, PF), axes=(1, 0)).astype(np.float32)


# ------------------------------------------------------------- bass program

_NC_CACHE = {}


def build_bass():
    if 0 in _NC_CACHE:
        return _NC_CACHE[0]
    import concourse.bass as bass
    import concourse.mybir as mybir
    from concourse.tile import TileContext
    F32 = mybir.dt.float32
    F16 = mybir.dt.float16

    nc = bass.Bass()
    d_state = nc.dram_tensor("state0", [C2, FTOT], F16, kind="ExternalInput")
    d_gates = nc.dram_tensor("gates", [C2, NMM * 3 * GW], F16, kind="ExternalInput")
    d_ident = nc.dram_tensor("ident", [C2, C2], F16, kind="ExternalInput")
    d_wread = nc.dram_tensor("wread", [C2, 4], F16, kind="ExternalInput")
    d_rout = nc.dram_tensor("rout", [4, PF], F32, kind="ExternalOutput")

    with TileContext(nc) as tc:
        with tc.tile_pool(name="const", bufs=1) as cpool, \
             tc.tile_pool(name="state", bufs=1) as spool, \
             tc.tile_pool(name="mm", bufs=3, space="PSUM") as mmp, \
             tc.tile_pool(name="tp", bufs=2, space="PSUM") as tpp:

            gts = cpool.tile([C2, NMM * 3 * GW], F16, tag="gates")
            ident = cpool.tile([C2, C2], F16, tag="ident")
            wread = cpool.tile([C2, 4], F16, tag="wread")
            # +128 tail: flip stationary reads [off:off+128] and may spill
            # past the live 6400 elems (garbage rows >=100 are never drained)
            stA_t = spool.tile([C2, FTOT + 128], F16, tag="stA")
            stB_t = spool.tile([C2, FTOT + 128], F16, tag="stB")
            stA = stA_t[:, 0:FTOT]
            stB = stB_t[:, 0:FTOT]
            ptile = spool.tile([C2, PF], F16, tag="probs")
            tmp16 = spool.tile([C2, PF], F16, tag="probs2")
            rtile = spool.tile([4, PF], F32, tag="rt")

            GSPLIT = 4 * 3 * GW   # first 4 gates load first
            nc.sync.dma_start(out=stA[:, :], in_=d_state[:, :])
            nc.scalar.dma_start(out=gts[:, 0:GSPLIT], in_=d_gates[:, 0:GSPLIT])
            nc.gpsimd.dma_start(out=ident[:, :], in_=d_ident[:, :])
            nc.gpsimd.dma_start(out=wread[:, :], in_=d_wread[:, :])
            nc.sync.dma_start(out=gts[:, GSPLIT:], in_=d_gates[:, GSPLIT:])

            drain_tgl = [0]

            def drain(dst_ap, src_ap):
                if drain_tgl[0] == 0:
                    nc.scalar.copy(out=dst_ap, in_=src_ap)
                else:
                    nc.vector.tensor_copy(dst_ap, src_ap)
                drain_tgl[0] ^= 1

            cur_t, nxt_t = stA_t, stB_t
            mi = 0
            for oi, op in enumerate(FULL_OPS):
                cur = cur_t[:, 0:FTOT]
                nxt = nxt_t[:, 0:FTOT]
                # 5-d views [p, fA, pl, fB, b]
                cur5 = cur.rearrange("p (fA pl fB b) -> p fA pl fB b",
                                     fA=CUT, pl=2, fB=CUT, b=BC)
                nxt5 = nxt.rearrange("p (fA pl fB b) -> p fA pl fB b",
                                     fA=CUT, pl=2, fB=CUT, b=BC)
                # alternate layout (pl, b, fA, fB) used between a gate and its
                # following flip: the (fA, fB) block is 100 contiguous elems,
                # so the flip matmul's stationary operand is a plain slice
                nxtF = nxt.rearrange("p (pl b fA fB) -> p pl b fA fB",
                                     pl=2, b=BC, fA=CUT, fB=CUT)
                pre_flip = (op == 'M' and oi + 1 < len(FULL_OPS)
                            and FULL_OPS[oi + 1] == 'F')
                if op == 'M':
                    swap = MM_SWAP[mi]
                    Ur = gts[:, mi * 3 * GW:mi * 3 * GW + GW]
                    nUi = gts[:, mi * 3 * GW + GW:mi * 3 * GW + 2 * GW]
                    Ui = gts[:, mi * 3 * GW + 2 * GW:mi * 3 * GW + 3 * GW]
                    mi += 1
                    for h in range(5):
                        # psum tiles span 2 banks: fA=2h at cols 0:320,
                        # fA=2h+1 at cols 512:832 (each within one bank)
                        psA = mmp.tile([128, 1024], F32, tag="mm")
                        psB = mmp.tile([128, 1024], F32, tag="mm")
                        rr = [cur5[:, 2 * h + k, 0, :, :] for k in range(2)]
                        ri = [cur5[:, 2 * h + k, 1, :, :] for k in range(2)]
                        # same-weight matmuls adjacent: 3 LDWEIGHTS per h
                        for k in range(2):
                            nc.tensor.matmul(psA[:, 512 * k:512 * k + 320], Ur,
                                             rr[k], start=True, stop=False)
                        for k in range(2):
                            nc.tensor.matmul(psB[:, 512 * k:512 * k + 320], Ur,
                                             ri[k], start=True, stop=False)
                        for k in range(2):
                            nc.tensor.matmul(psA[:, 512 * k:512 * k + 320], nUi,
                                             ri[k], start=False, stop=True)
                        for k in range(2):
                            nc.tensor.matmul(psB[:, 512 * k:512 * k + 320], Ui,
                                             rr[k], start=False, stop=True)
                        for pl, ps in ((0, psA), (1, psB)):
                            src = ps[0:C2, :].rearrange("p (k r) -> p k r",
                                                        k=2, r=512)[:, :, 0:320] \
                                .rearrange("p k (j b) -> p k j b", j=CUT, b=BC)
                            if pre_flip:
                                # L*2 layout: dst iter (k, b, j)
                                src = ps[0:C2, :].rearrange("p (k r) -> p k r",
                                                            k=2, r=512)[:, :, 0:320] \
                                    .rearrange("p k (j b) -> p k b j", j=CUT, b=BC)
                                dst = nxtF[:, pl, :, 2 * h:2 * h + 2, :] \
                                    .rearrange("p b k j -> p k b j")
                            elif not swap:
                                dst = nxt5[:, 2 * h:2 * h + 2, pl, :, :]
                            else:
                                # psum enum (k2, j10, b) -> dst fA=j, fB=2h+k
                                dst = nxt5[:, :, pl, 2 * h:2 * h + 2, :] \
                                    .rearrange("p j i b -> p i j b")
                            drain(dst, src)
                elif op == 'S':
                    # 3-queue spread; chunk h of the next gate needs v=2h,2h+1,
                    # so early pairs go to the fastest queue (gpsimd SWDGE)
                    s_engs = [nc.gpsimd, nc.gpsimd, nc.sync, nc.sync,
                              nc.scalar, nc.scalar, nc.gpsimd, nc.gpsimd,
                              nc.sync, nc.scalar]
                    for v in range(CUT):
                        s_engs[v].dma_start(
                            out=nxt[:, v * 640:(v + 1) * 640],
                            in_=cur[v * CUT:(v + 1) * CUT, :].rearrange(
                                "s (u r) -> s u r", u=CUT, r=640))
                else:  # F
                    # flip via REGULAR matmul: stationary = contiguous 128-col
                    # slice of the L*2 state (100 live cols = (fA,fB) block),
                    # moving = fp16 identity.  out = slice.T @ I, partitions
                    # 100..127 are spill garbage and are never drained.
                    for pl in range(2):
                        for bq in range(BC // 4):
                            pt = tpp.tile([128, 400], F32, tag="tp")
                            for q in range(4):
                                bb = bq * 4 + q
                                off = pl * 3200 + bb * 100
                                lhsT = cur_t[:, off:off + 128]
                                nc.tensor.matmul(pt[:, q * 100:(q + 1) * 100],
                                                 lhsT, ident[:, :],
                                                 start=True, stop=True)
                            # psum enum (q, p1, p2) -> dst fA=p2, fB=p1,
                            # b=bq*4+q; iterate (p2, p1, q) so dst inner is
                            # the contiguous b-quad
                            dst = nxt5[:, :, pl, :, bq * 4:bq * 4 + 4]
                            src = pt[0:C2, :].rearrange("p (q p1 p2) -> p p2 p1 q",
                                                        q=4, p1=CUT, p2=CUT)
                            drain(dst, src)
                cur_t, nxt_t = nxt_t, cur_t

            # readout: P = re^2 + im^2
            cur = cur_t[:, 0:FTOT]
            cur5 = cur.rearrange("p (fA pl fB b) -> p fA pl fB b",
                                 fA=CUT, pl=2, fB=CUT, b=BC)
            re_ap = cur5[:, :, 0, :, :]
            im_ap = cur5[:, :, 1, :, :]
            pt3 = ptile[:, :].rearrange("p (fA fB b) -> p fA fB b",
                                        fA=CUT, fB=CUT, b=BC)
            tm3 = tmp16[:, :].rearrange("p (fA fB b) -> p fA fB b",
                                        fA=CUT, fB=CUT, b=BC)
            nc.vector.tensor_mul(pt3, re_ap, re_ap)
            nc.vector.tensor_mul(tm3, im_ap, im_ap)
            nc.vector.tensor_add(ptile[:, :], ptile[:, :], tmp16[:, :])
            for n in range(PF // 400):
                pr = tpp.tile([4, 400], F32, tag="tp")
                nc.tensor.matmul(pr[:, :], wread[:, :], ptile[:, n * 400:(n + 1) * 400],
                                 start=True, stop=True)
                drain(rtile[:, n * 400:(n + 1) * 400], pr[:, :])
            nc.sync.dma_start(out=d_rout[:, :], in_=rtile[:, :])

    nc.finalize()
    _legalize_waits(nc)
    _NC_CACHE[0] = nc
    return nc


def _legalize_waits(nc):
    """This walrus build encodes at most ONE sync wait per instruction.
    Split any instruction with N>1 waits into (N-1) preceding single-wait
    NoOps on the same engine (engines execute in order, so sequential
    waits are equivalent to simultaneous ones)."""
    import copy
    import concourse.mybir as mybir
    m = nc.m
    new_module = copy.replace(m, functions=[])
    nsplit = [0]
    for function in m.functions:
        new_function = copy.replace(function, blocks=[])
        new_function.set_allocations_from_list(function.allocations)
        for block in function.blocks:
            new_insts = []
            for inst in block.instructions:
                si = inst.sync_info
                if si is not None and si.on_wait and len(si.on_wait) > 1:
                    waits = list(si.on_wait)
                    for k, w in enumerate(waits[:-1]):
                        new_insts.append(mybir.InstNoOp(
                            name=f"{inst.name}-lw{k}",
                            engine=inst.engine,
                            sync_info=mybir.SyncInfo(on_wait=[w], on_update=[]),
                            bass_nofuse=True,
                        ))
                    inst.sync_info = mybir.SyncInfo(
                        on_wait=[waits[-1]], on_update=list(si.on_update))
                    nsplit[0] += 1
                new_insts.append(inst)
            new_function.blocks.append(copy.replace(block, instructions=new_insts))
        new_module.functions.append(new_function)
    nc.m = new_module
    return nsplit[0]


def kernel(inputs, cvqnn_weights, batch_size):
    inputs = np.asarray(inputs)
    assert inputs.shape[0] == int(batch_size) == B
    mms = build_gates(np.asarray(cvqnn_weights))
    st = initial_state_dev(inputs)
    gd = gates_dram(mms)

    nc = build_bass()
    from concourse.bass_utils import run_bass_kernel_spmd
    in_maps = make_in_maps(st, gd)
    res = run_bass_kernel_spmd(nc, in_maps, core_ids=list(range(NCORES)))
    routs = [res.results[c]["rout"] for c in range(NCORES)]
    return assemble_output(routs)


# revision 26
# speedup vs baseline: 1.0141x; 1.0141x over previous
"""CVQNN batched policy forward on 8 Trainium2 NeuronCores (fp16 v2).

Math: B=256 4-mode Fock states (cutoff 10) through 4 CVQNN layers.
Device layout per core: state [100 partitions = mode pair (maj,min),
6400 free = fA*640 + plane*320 + fB*32 + b], fp16, complex as separate
re/im planes nested INSIDE each fA block so a pair-digit shuffle is a
contiguous 640-element move.

Every gate is a 100x100 complex matmul on the partition-side mode pair
(4 fp16 matmuls with PSUM accumulation, 640-wide chunks, 3 LDWEIGHTS
per gate via weight-reuse ordering). Diagonal/single-mode gates are
folded into pair gates on the host. Pair-layout changes: 'S' = 10
SBUF->SBUF DMAs (1280B descriptors), 'F' = PE transposes. Batch is
data-parallel over 8 cores.
"""
import numpy as np

B, D, CUT, LAYERS, NCORES = 256, 4, 10, 4, 8
K = D * (D - 1) // 2
C2 = CUT * CUT
BC = B // NCORES            # 32 batch per core
# free strides (elements): fA*640 + plane*320 + fB*32 + b
FTOT = 2 * C2 * BC          # 6400 free elems per state tile
PF = C2 * BC                # 3200 elems per plane (logical)
NMM = 12 * LAYERS + 1       # 49 pair-gate matmuls

# ---------------------------------------------------------------- host math

def _ops():
    a = np.diag(np.sqrt(np.arange(1, CUT)), k=1).astype(np.complex128)
    return a, a.conj().T.copy(), np.arange(CUT, dtype=np.float64)


def _expm_antiherm(G):
    w, V = np.linalg.eigh(-1j * G)
    return (V * np.exp(1j * w)) @ V.conj().T


def _bs(a, adag, t, p):
    Aab = np.kron(adag, a)
    return _expm_antiherm(t * (np.exp(1j * p) * Aab - np.exp(-1j * p) * Aab.conj().T))


def _orient(M, pair, in_order, out_order):
    """M acts canonically on index m_x*10+m_y for pair=(x,y).  Reindex for
    input digit order in_order and output digit order out_order."""
    M4 = M.reshape(CUT, CUT, CUT, CUT)  # [ox, oy, ix, iy]
    perm = [0, 1, 2, 3]
    if tuple(out_order) != tuple(pair):
        perm[0], perm[1] = perm[1], perm[0]
    if tuple(in_order) != tuple(pair):
        perm[2], perm[3] = perm[3], perm[2]
    return M4.transpose(perm).reshape(C2, C2)


def build_gates(cvqnn_weights):
    """Returns mm_list where entries are (matrix[100x100 complex],
    swap_free: bool). Layout walk is fixed."""
    a, adag, n = _ops()
    I10 = np.eye(CUT, dtype=np.complex128)
    w = np.asarray(cvqnn_weights, np.float64)
    PAIRS = [(0, 1), (0, 2), (0, 3), (1, 2), (1, 3), (2, 3)]

    mms = []
    fold01 = np.eye(C2, dtype=np.complex128)
    fold2 = I10.copy()
    fold3 = I10.copy()
    for l in range(LAYERS):
        o = 0
        th1 = w[l, o:o + K]; o += K
        ph1 = w[l, o:o + K]; o += K
        vp1 = w[l, o:o + D]; o += D
        rsq = w[l, o:o + D]; o += D
        th2 = w[l, o:o + K]; o += K
        ph2 = w[l, o:o + K]; o += K
        vp2 = w[l, o:o + D]; o += D
        rd = w[l, o:o + D]; o += D
        phd = w[l, o:o + D]; o += D
        kap = w[l, o:o + D]
        U = {PAIRS[q]: _bs(a, adag, th1[q], ph1[q]) for q in range(K)}
        V = {PAIRS[q]: _bs(a, adag, th2[q], ph2[q]) for q in range(K)}
        S = [_expm_antiherm(0.5 * rsq[m] * (a @ a - adag @ adag)) for m in range(D)]
        al = rd * np.exp(1j * phd)
        Dm = [_expm_antiherm(al[m] * adag - np.conj(al[m]) * a) for m in range(D)]
        P1 = [np.diag(np.exp(1j * vp1[m] * n)) for m in range(D)]
        P2 = [np.diag(np.exp(1j * vp2[m] * n)) for m in range(D)]
        Km = [np.diag(np.exp(1j * kap[m] * n * n)) for m in range(D)]
        SQ01 = np.kron(S[0] @ P1[0], S[1] @ P1[1])
        SQ23 = np.kron(S[2] @ P1[2], S[3] @ P1[3])
        DP01 = np.kron(Dm[0] @ P2[0], Dm[1] @ P2[1])
        DP23 = np.kron(Dm[2] @ P2[2], Dm[3] @ P2[3])

        mms.append((_orient(U[(0, 1)] @ fold01, (0, 1), (1, 0), (1, 0)), False))
        mms.append((_orient(U[(0, 2)] @ np.kron(I10, fold2), (0, 2), (0, 2), (2, 0)), True))
        mms.append((_orient(U[(0, 3)] @ np.kron(I10, fold3), (0, 3), (0, 3), (0, 3)), False))
        mms.append((_orient(U[(1, 2)], (1, 2), (2, 1), (2, 1)), False))
        mms.append((_orient(U[(1, 3)], (1, 3), (1, 3), (1, 3)), False))
        mms.append((_orient(SQ23 @ U[(2, 3)], (2, 3), (3, 2), (3, 2)), False))
        mms.append((_orient(V[(0, 1)] @ SQ01, (0, 1), (1, 0), (1, 0)), False))
        mms.append((_orient(V[(0, 2)], (0, 2), (0, 2), (2, 0)), True))
        mms.append((_orient(V[(0, 3)], (0, 3), (0, 3), (0, 3)), False))
        mms.append((_orient(V[(1, 2)], (1, 2), (2, 1), (2, 1)), False))
        mms.append((_orient(V[(1, 3)], (1, 3), (1, 3), (1, 3)), False))
        mms.append((_orient(DP23 @ V[(2, 3)], (2, 3), (3, 2), (3, 2)), False))
        if l < LAYERS - 1:
            fold01 = np.kron(Km[0], Km[1]) @ DP01
            fold2 = Km[2]
            fold3 = Km[3]
        else:
            mms.append((_orient(DP01, (0, 1), (1, 0), (1, 0)), False))
    assert len(mms) == NMM
    return mms


# op schedule per layer: 'M' gate, 'S' shuffle, 'F' flip
LAYER_OPS = ['M', 'S', 'M', 'S', 'M', 'F', 'M', 'S', 'M', 'S', 'M', 'F',
             'M', 'S', 'M', 'S', 'M', 'F', 'M', 'S', 'M', 'S', 'M', 'F']
FULL_OPS = LAYER_OPS * LAYERS + ['M']

MM_SWAP = []
for _l in range(LAYERS):
    MM_SWAP += [False, True, False, False, False, False,
                False, True, False, False, False, False]
MM_SWAP.append(False)


def initial_state_dev(inputs):
    """Per-core device state tiles [NCORES, 100, 6400] fp16 in layout
    [1,0 | 2,3]: p=m1*10+m0, f=fA(m2)*640 + plane*320 + fB(m3)*32 + b."""
    a, adag, n = _ops()
    z = 0.5j
    S0 = _expm_antiherm(0.5 * (np.conj(z) * (a @ a) - z * (adag @ adag)))
    psi0 = S0[:, 0]
    r = np.asarray(inputs, np.float64).reshape(-1)
    wv, Vx = np.linalg.eigh(-1j * (adag - a))
    w0 = Vx.conj().T @ psi0
    psi = (np.exp(1j * np.outer(r, wv)) * w0[None, :]) @ Vx.T
    psi = psi.reshape(B, D, CUT)
    st = np.einsum('bi,bj,bk,bl->bijkl', psi[:, 0], psi[:, 1], psi[:, 2], psi[:, 3])
    # [b, m0,m1,m2,m3] -> p=(m1,m0), fA=m2, fB=m3
    st = st.transpose(2, 1, 3, 4, 0).reshape(C2, CUT, CUT, B)  # [p, fA, fB, b]
    out = np.empty((NCORES, C2, CUT, 2, CUT, BC), np.float32)
    for c in range(NCORES):
        blk = st[:, :, :, c * BC:(c + 1) * BC]  # [100, 10, 10, 32]
        out[c, :, :, 0, :, :] = blk.real
        out[c, :, :, 1, :, :] = blk.imag
    return out.reshape(NCORES, C2, FTOT).astype(np.float16)


def readout_weights():
    """lhsT [100, 4] fp16 for the device readout matmul, layout [1,0|2,3]."""
    n = np.arange(CUT, dtype=np.float32)
    Wt = np.zeros((C2, 4), np.float32)
    for p in range(C2):
        Wt[p, 0] = n[p % 10]    # mode 0 (partition minor)
        Wt[p, 1] = n[p // 10]   # mode 1 (partition major)
        Wt[p, 2] = 1.0
        Wt[p, 3] = 1.0
    return Wt.astype(np.float16)


def assemble_output(routs):
    """routs: [NCORES, 4, 3200] f32 (free = fA*320 + fB*32 + b) -> [B, 4]."""
    n = np.arange(CUT, dtype=np.float64)
    out = np.zeros((B, D), np.float64)
    for c in range(NCORES):
        R = np.asarray(routs[c], np.float64).reshape(4, CUT, CUT, BC)
        sl = slice(c * BC, (c + 1) * BC)
        out[sl, 0] = R[0].sum(axis=(0, 1))
        out[sl, 1] = R[1].sum(axis=(0, 1))
        out[sl, 2] = (R[2] * n[:, None, None]).sum(axis=(0, 1))   # weight by fA=m2
        out[sl, 3] = (R[3] * n[None, :, None]).sum(axis=(0, 1))   # weight by fB=m3
    return out.astype(np.float32)


GW = 128   # gate stationary padded to 128 cols (fast weight load)


def gates_dram(mms):
    """[100, NMM*3*GW] fp16: per gate UrT | (-Ui)T | UiT, each zero-padded
    to 128 columns so LDWEIGHTS takes the fast path."""
    g = np.zeros((C2, NMM * 3 * GW), np.float32)
    for i, (M, _) in enumerate(mms):
        g[:, i * 3 * GW:i * 3 * GW + C2] = M.real.T.astype(np.float32)
        g[:, i * 3 * GW + GW:i * 3 * GW + GW + C2] = (-M.imag.T).astype(np.float32)
        g[:, i * 3 * GW + 2 * GW:i * 3 * GW + 2 * GW + C2] = M.imag.T.astype(np.float32)
    return g.astype(np.float16)


def make_in_maps(st, gd):
    ident = np.eye(C2, dtype=np.float16)
    wr = readout_weights()
    return [{"state0": st[c], "gates": gd, "ident": ident, "wread": wr}
            for c in range(NCORES)]


# ------------------------------------------------------------ numpy dev-sim

def dev_sim(state_core, mms):
    """Numpy model of the device op stream for one core (fp16 rounding).
    state_core: [100, 6400] f16. Returns R [4, 3200] f32."""
    f64 = np.float64
    rnd = lambda x: x.astype(np.float16).astype(f64)
    X = state_core.astype(f64).reshape(C2, CUT, 2, CUT, BC)  # [p, fA, pl, fB, b]
    mi = 0
    for op in FULL_OPS:
        if op == 'M':
            M, swap = mms[mi]; mi += 1
            Mr = rnd(M.real); Mi = rnd(M.imag)
            re = X[:, :, 0]; im = X[:, :, 1]          # [p, fA, fB, b]
            re2 = np.tensordot(Mr, re, axes=(1, 0)) - np.tensordot(Mi, im, axes=(1, 0))
            im2 = np.tensordot(Mr, im, axes=(1, 0)) + np.tensordot(Mi, re, axes=(1, 0))
            re2 = rnd(re2); im2 = rnd(im2)
            if swap:
                re2 = re2.transpose(0, 2, 1, 3)
                im2 = im2.transpose(0, 2, 1, 3)
            X = np.stack([re2, im2], axis=2).copy()
        elif op == 'S':
            # [v,s | u, pl, w, b] -> [s,u | v, pl, w, b]
            X6 = X.reshape(CUT, CUT, CUT, 2, CUT, BC)  # [v, s, u, pl, w, b]
            X = X6.transpose(1, 2, 0, 3, 4, 5).reshape(C2, CUT, 2, CUT, BC)
        else:  # F
            # [p1,p2 | f1, pl, f2, b] -> [f1,f2 | p2, pl, p1, b]
            X6 = X.reshape(CUT, CUT, CUT, 2, CUT, BC)  # [p1, p2, f1, pl, f2, b]
            X = X6.transpose(2, 4, 1, 3, 0, 5).reshape(C2, CUT, 2, CUT, BC)
    P = X[:, :, 0] ** 2 + X[:, :, 1] ** 2              # [p, fA, fB, b]
    Wt = readout_weights().astype(f64)
    return np.tensordot(Wt.T, P.reshape(C2, PF), axes=(1, 0)).astype(np.float32)


# ------------------------------------------------------------- bass program

_NC_CACHE = {}


def build_bass():
    if 0 in _NC_CACHE:
        return _NC_CACHE[0]
    import concourse.bass as bass
    import concourse.mybir as mybir
    from concourse.tile import TileContext
    F32 = mybir.dt.float32
    F16 = mybir.dt.float16

    nc = bass.Bass()
    d_state = nc.dram_tensor("state0", [C2, FTOT], F16, kind="ExternalInput")
    d_gates = nc.dram_tensor("gates", [C2, NMM * 3 * GW], F16, kind="ExternalInput")
    d_ident = nc.dram_tensor("ident", [C2, C2], F16, kind="ExternalInput")
    d_wread = nc.dram_tensor("wread", [C2, 4], F16, kind="ExternalInput")
    d_rout = nc.dram_tensor("rout", [4, PF], F32, kind="ExternalOutput")

    with TileContext(nc) as tc:
        with tc.tile_pool(name="const", bufs=1) as cpool, \
             tc.tile_pool(name="state", bufs=1) as spool, \
             tc.tile_pool(name="mm", bufs=3, space="PSUM") as mmp, \
             tc.tile_pool(name="tp", bufs=2, space="PSUM") as tpp:

            NG0 = 4   # first gates in their own tile so gate 1 starts early
            gts0 = cpool.tile([C2, NG0 * 3 * GW], F16, tag="gates0")
            gts1 = cpool.tile([C2, (NMM - NG0) * 3 * GW], F16, tag="gates1")
            ident = cpool.tile([C2, C2], F16, tag="ident")
            wread = cpool.tile([C2, 4], F16, tag="wread")
            # +128 tail: flip stationary reads [off:off+128] and may spill
            # past the live 6400 elems (garbage rows >=100 are never drained)
            stA_t = spool.tile([C2, FTOT + 128], F16, tag="stA")
            stB_t = spool.tile([C2, FTOT + 128], F16, tag="stB")
            stA = stA_t[:, 0:FTOT]
            stB = stB_t[:, 0:FTOT]
            ptile = spool.tile([C2, PF], F16, tag="probs")
            tmp16 = spool.tile([C2, PF], F16, tag="probs2")
            rtile = spool.tile([4, PF], F32, tag="rt")

            GSPLIT = NG0 * 3 * GW
            nc.sync.dma_start(out=stA[:, :], in_=d_state[:, :])
            nc.scalar.dma_start(out=gts0[:, :], in_=d_gates[:, 0:GSPLIT])
            nc.gpsimd.dma_start(out=ident[:, :], in_=d_ident[:, :])
            nc.gpsimd.dma_start(out=wread[:, :], in_=d_wread[:, :])
            nc.scalar.dma_start(out=gts1[:, :], in_=d_gates[:, GSPLIT:])

            def gate_w(gi):
                if gi < NG0:
                    t, o = gts0, gi * 3 * GW
                else:
                    t, o = gts1, (gi - NG0) * 3 * GW
                return (t[:, o:o + GW], t[:, o + GW:o + 2 * GW],
                        t[:, o + 2 * GW:o + 3 * GW])

            drain_tgl = [0]

            def drain(dst_ap, src_ap):
                if drain_tgl[0] == 0:
                    nc.scalar.copy(out=dst_ap, in_=src_ap)
                else:
                    nc.vector.tensor_copy(dst_ap, src_ap)
                drain_tgl[0] ^= 1

            cur_t, nxt_t = stA_t, stB_t
            mi = 0
            for oi, op in enumerate(FULL_OPS):
                cur = cur_t[:, 0:FTOT]
                nxt = nxt_t[:, 0:FTOT]
                # 5-d views [p, fA, pl, fB, b]
                cur5 = cur.rearrange("p (fA pl fB b) -> p fA pl fB b",
                                     fA=CUT, pl=2, fB=CUT, b=BC)
                nxt5 = nxt.rearrange("p (fA pl fB b) -> p fA pl fB b",
                                     fA=CUT, pl=2, fB=CUT, b=BC)
                # alternate layout (pl, b, fA, fB) used between a gate and its
                # following flip: the (fA, fB) block is 100 contiguous elems,
                # so the flip matmul's stationary operand is a plain slice
                nxtF = nxt.rearrange("p (pl b fA fB) -> p pl b fA fB",
                                     pl=2, b=BC, fA=CUT, fB=CUT)
                pre_flip = (op == 'M' and oi + 1 < len(FULL_OPS)
                            and FULL_OPS[oi + 1] == 'F')
                if op == 'M':
                    swap = MM_SWAP[mi]
                    Ur, nUi, Ui = gate_w(mi)
                    mi += 1
                    # process chunks in S-DMA arrival order (pairs 0,2,4 on
                    # the faster gpsimd queue, 1,3 on sync)
                    for h in (0, 1, 2, 4, 3):
                        # psum tiles span 2 banks: fA=2h at cols 0:320,
                        # fA=2h+1 at cols 512:832 (each within one bank)
                        psA = mmp.tile([128, 1024], F32, tag="mm")
                        psB = mmp.tile([128, 1024], F32, tag="mm")
                        rr = [cur5[:, 2 * h + k, 0, :, :] for k in range(2)]
                        ri = [cur5[:, 2 * h + k, 1, :, :] for k in range(2)]
                        # same-weight matmuls adjacent: 3 LDWEIGHTS per h
                        for k in range(2):
                            nc.tensor.matmul(psA[:, 512 * k:512 * k + 320], Ur,
                                             rr[k], start=True, stop=False)
                        for k in range(2):
                            nc.tensor.matmul(psB[:, 512 * k:512 * k + 320], Ur,
                                             ri[k], start=True, stop=False)
                        for k in range(2):
                            nc.tensor.matmul(psA[:, 512 * k:512 * k + 320], nUi,
                                             ri[k], start=False, stop=True)
                        for k in range(2):
                            nc.tensor.matmul(psB[:, 512 * k:512 * k + 320], Ui,
                                             rr[k], start=False, stop=True)
                        for pl, ps in ((0, psA), (1, psB)):
                            src = ps[0:C2, :].rearrange("p (k r) -> p k r",
                                                        k=2, r=512)[:, :, 0:320] \
                                .rearrange("p k (j b) -> p k j b", j=CUT, b=BC)
                            if pre_flip:
                                # L*2 layout: dst iter (k, b, j)
                                src = ps[0:C2, :].rearrange("p (k r) -> p k r",
                                                            k=2, r=512)[:, :, 0:320] \
                                    .rearrange("p k (j b) -> p k b j", j=CUT, b=BC)
                                dst = nxtF[:, pl, :, 2 * h:2 * h + 2, :] \
                                    .rearrange("p b k j -> p k b j")
                            elif not swap:
                                dst = nxt5[:, 2 * h:2 * h + 2, pl, :, :]
                            else:
                                # psum enum (k2, j10, b) -> dst fA=j, fB=2h+k
                                dst = nxt5[:, :, pl, 2 * h:2 * h + 2, :] \
                                    .rearrange("p j i b -> p i j b")
                            drain(dst, src)
                elif op == 'S':
                    # chunk h of the next gate needs v=2h,2h+1: pairs 0,2,4 on
                    # the faster gpsimd queue, pairs 1,3 on sync
                    s_engs = {0: nc.gpsimd, 1: nc.gpsimd, 2: nc.sync,
                              3: nc.sync, 4: nc.gpsimd, 5: nc.gpsimd,
                              6: nc.sync, 7: nc.sync, 8: nc.gpsimd,
                              9: nc.gpsimd}
                    for v in (0, 1, 2, 3, 4, 5, 6, 7, 8, 9):
                        s_engs[v].dma_start(
                            out=nxt[:, v * 640:(v + 1) * 640],
                            in_=cur[v * CUT:(v + 1) * CUT, :].rearrange(
                                "s (u r) -> s u r", u=CUT, r=640))
                else:  # F
                    # flip via REGULAR matmul: stationary = contiguous 128-col
                    # slice of the L*2 state (100 live cols = (fA,fB) block),
                    # moving = fp16 identity.  out = slice.T @ I, partitions
                    # 100..127 are spill garbage and are never drained.
                    for pl in range(2):
                        for bq in range(BC // 4):
                            pt = tpp.tile([128, 400], F32, tag="tp")
                            for q in range(4):
                                bb = bq * 4 + q
                                off = pl * 3200 + bb * 100
                                lhsT = cur_t[:, off:off + 128]
                                nc.tensor.matmul(pt[:, q * 100:(q + 1) * 100],
                                                 lhsT, ident[:, :],
                                                 start=True, stop=True)
                            # psum enum (q, p1, p2) -> dst fA=p2, fB=p1,
                            # b=bq*4+q; iterate (p2, p1, q) so dst inner is
                            # the contiguous b-quad
                            dst = nxt5[:, :, pl, :, bq * 4:bq * 4 + 4]
                            src = pt[0:C2, :].rearrange("p (q p1 p2) -> p p2 p1 q",
                                                        q=4, p1=CUT, p2=CUT)
                            drain(dst, src)
                cur_t, nxt_t = nxt_t, cur_t

            # readout: P = re^2 + im^2
            cur = cur_t[:, 0:FTOT]
            cur5 = cur.rearrange("p (fA pl fB b) -> p fA pl fB b",
                                 fA=CUT, pl=2, fB=CUT, b=BC)
            re_ap = cur5[:, :, 0, :, :]
            im_ap = cur5[:, :, 1, :, :]
            pt3 = ptile[:, :].rearrange("p (fA fB b) -> p fA fB b",
                                        fA=CUT, fB=CUT, b=BC)
            tm3 = tmp16[:, :].rearrange("p (fA fB b) -> p fA fB b",
                                        fA=CUT, fB=CUT, b=BC)
            nc.vector.tensor_mul(pt3, re_ap, re_ap)
            nc.vector.tensor_mul(tm3, im_ap, im_ap)
            nc.vector.tensor_add(ptile[:, :], ptile[:, :], tmp16[:, :])
            for n in range(PF // 400):
                pr = tpp.tile([4, 400], F32, tag="tp")
                nc.tensor.matmul(pr[:, :], wread[:, :], ptile[:, n * 400:(n + 1) * 400],
                                 start=True, stop=True)
                drain(rtile[:, n * 400:(n + 1) * 400], pr[:, :])
            nc.sync.dma_start(out=d_rout[:, :], in_=rtile[:, :])

    nc.finalize()
    _legalize_waits(nc)
    _NC_CACHE[0] = nc
    return nc


def _legalize_waits(nc):
    """This walrus build encodes at most ONE sync wait per instruction.
    Split any instruction with N>1 waits into (N-1) preceding single-wait
    NoOps on the same engine (engines execute in order, so sequential
    waits are equivalent to simultaneous ones)."""
    import copy
    import concourse.mybir as mybir
    m = nc.m
    new_module = copy.replace(m, functions=[])
    nsplit = [0]
    for function in m.functions:
        new_function = copy.replace(function, blocks=[])
        new_function.set_allocations_from_list(function.allocations)
        for block in function.blocks:
            new_insts = []
            for inst in block.instructions:
                si = inst.sync_info
                if si is not None and si.on_wait and len(si.on_wait) > 1:
                    waits = list(si.on_wait)
                    for k, w in enumerate(waits[:-1]):
                        new_insts.append(mybir.InstNoOp(
                            name=f"{inst.name}-lw{k}",
                            engine=inst.engine,
                            sync_info=mybir.SyncInfo(on_wait=[w], on_update=[]),
                            bass_nofuse=True,
                        ))
                    inst.sync_info = mybir.SyncInfo(
                        on_wait=[waits[-1]], on_update=list(si.on_update))
                    nsplit[0] += 1
                new_insts.append(inst)
            new_function.blocks.append(copy.replace(block, instructions=new_insts))
        new_module.functions.append(new_function)
    nc.m = new_module
    return nsplit[0]


def kernel(inputs, cvqnn_weights, batch_size):
    inputs = np.asarray(inputs)
    assert inputs.shape[0] == int(batch_size) == B
    mms = build_gates(np.asarray(cvqnn_weights))
    st = initial_state_dev(inputs)
    gd = gates_dram(mms)

    nc = build_bass()
    from concourse.bass_utils import run_bass_kernel_spmd
    in_maps = make_in_maps(st, gd)
    res = run_bass_kernel_spmd(nc, in_maps, core_ids=list(range(NCORES)))
    routs = [res.results[c]["rout"] for c in range(NCORES)]
    return assemble_output(routs)
